# revision 69
# baseline (speedup 1.0000x reference)
"""Trainium2 Bass kernel for nn_MultiHeadAttention_88330297410289.

Full-input contract: kernel(**inputs) takes the complete tensors
(hidden_states [32,256,2048], Wq/Wk/Wv/Wo [2048,2048], all fp32) and
returns the full output [32,256,2048] fp32.

Measured: 494,557 ns, absmax rel err 0.0156 (gate 2e-2). Baseline
(all-bf16, same schedule minus the fp8 V tail) was 509,982 ns — that
one already sits ON the bf16 PE roofline (~2.36 GHz effective, 1.0
moving-column/cycle), so the gain comes from cutting PE cycles with
fp8 DoubleRow plus gap elimination, not from more overlap.

Strategy: data-parallel over the batch dim across 8 NeuronCores
(4 batches = 1024 tokens per core, no collectives). Per core, all
activations live in transposed [feature, token] layout so every matmul
streams directly from SBUF with no on-chip transposes:

  qT = WqT.T-contract(xT)    (per head-column block, PSUM [128, 512])
  RoPE: rq via SBUF->SBUF partition-shift DMAs,
        q' = qT*cos + rq*sin on DVE (scale 1/sqrt(hd) folded into q tables)
  scoresT[sk,sq] = k'T.T-contract(q'T) per (batch, head)
  expT = exp(scoresT) on ACT (single [128,512] op per batch)
  sums broadcast over partitions via all-ones matmul; reciprocal on DVE
  outT_un[d,sq] = v.T-contract(expT); normalize on DVE -> outT
  y = outT.T-contract(WoT)   (natural [token, feature] output layout)

Matmuls run in bf16 (fp32 PSUM accumulation); weights/x are cast
host-side; y returns as bf16 and is upcast host-side. The 1/sqrt(hd)
attention scale is folded into Wq host-side.

fp8 DoubleRow on the V projection: k-tiles 0..3 of the V contraction
run as two e4m3 DoubleRow matmuls (2 k-tiles per 512-cycle matmul —
2x the bf16 flop rate), closing each PSUM group after the 12 bf16
k-tiles. Scaling dodges the HW fp8 subnormal flush AND keeps one PSUM
scale: wv(bf16) is x4 host-side, the DR term is (8*W)*(x/2), and the
ACT evacuation rescales by 1/4. x8 = e4m3(xt/2) is cast ON-CHIP by
the otherwise-idle DVE (no extra DMA in the bandwidth-critical ov=0
window); wv8 slices ([P, KF, 512], slice-contiguous DRAM layout) ride
the sync-ring tail. This cuts 2 of 16 k-tile units per V group
(-16K PE cycles) at ~+0.9%% absmax error (fp8 noise dilutes through
the attention average — the same trick on the O projection FAILS:
peaked-attention rows give heavy-tailed, row-correlated error).
NOTE: fp8 sums/attnV/scores were all tried and are numerically dead
(exp dynamic range 1e-2..4.5e3 doesn't fit e4m3 with FTZ; e5m2's
2-bit mantissa mis-sums the dominant terms; q/k/ebf/v fp8 noise all
exceed the budget). DoubleRow on TRN2 is 2x bf16 flops (measured),
NOT the 4x the v2 cost model claims; both bf16 and fp8-DR stream
2 moving bytes/partition/cycle, so compensated hi/lo fp8 schemes
cannot beat bf16.

Scheduling: the two HWDGE rings are split by role — the sync ring
carries the wv quad stream, RoPE partition-shift DMAs, wo slice-0 and
y stores, while the scalar ring carries xt, per-head wq/wk prefetched
one head ahead, and later wo slices — so weight prefetch is never
head-of-line blocked behind a DMA that waits on compute. Weight
streams use [P, 4, 512] quad DMAs (one DGE issue per 4 tiles) to stay
under the ~600 ns/issue sequencer rate. Attention for head h-1 is
interleaved INTO head h's projection groups (scores after q-proj,
sums+attnV split around the k-proj groups) so the PE never waits on
the exp/reciprocal chain. The LAST head's attention is likewise
interleaved into the first O-projection slice (tails for batch b
unblock o-groups tt=2b,2b+1), with the O PSUM groups drawing from the
qkps pool (same tile name => same 4 banks) so all pools fit in 8
banks. V evacuations are ACT-only (a DVE copy there can queue behind
head-0 rope ops and stall the PSUM-bank reuse at the phase boundary).
PSUM: 8 banks = qkps 4 (shared with O) + scores 2 + acc 2; V phase
uses all 8 for pv, ov=0 i-outer (xt streams), ov>=1 tt-outer.
CAUTION: tile reads must FOLLOW their writes in program order (the
tracker gives no wait to a read issued before the write — the x8
casts must sit after every xt DMA issue).
"""

import numpy as np
import ml_dtypes

bf16 = ml_dtypes.bfloat16

# Problem shape (hardcoded per contract)
B, S, H = 32, 256, 2048
NH, HD = 16, 128
N_CORES = 8
B_LOC = B // N_CORES          # 4 batches per core
T = B_LOC * S                 # 1024 tokens per core
P = 128
KF = 4                        # leading k-tiles of the V proj done in fp8 DR

_CACHE = {}


def _rope_tables_np(seq_len, head_dim):
    inv_freq = 1.0 / (10000.0 ** (np.arange(0, head_dim, 2, dtype=np.float32) / head_dim))
    t = np.arange(seq_len, dtype=np.float32)
    freqs = np.einsum("i,j->ij", t, inv_freq).astype(np.float32)   # [s, d/2]
    emb = np.concatenate([freqs, freqs], axis=-1)                   # [s, d]
    return np.cos(emb).astype(np.float32), np.sin(emb).astype(np.float32)


def build_nc(nh=NH, t_tok=T, h_dim=H, b_loc=B_LOC, s_len=S):
    import concourse.tile as tile
    from concourse import bacc, mybir
    import bass_rust

    AF = bass_rust.ActivationFunctionType
    from concourse.alu_op_type import AluOpType

    assert nh * HD == h_dim
    IT = h_dim // P               # contraction i-tiles (16)
    TT = t_tok // P               # token 128-tiles (8)
    TS = t_tok // 512             # token 512-slices (2)
    OS = h_dim // 512             # feature 512-slices (4)
    SK = s_len // P               # key 128-tiles per batch (2)
    f32 = mybir.dt.float32
    bft = mybir.dt.bfloat16

    nc = bacc.Bacc("TRN2", target_bir_lowering=False, debug=False, num_devices=N_CORES)

    f8 = mybir.dt.float8e4
    DRMODE = mybir.MatmulPerfMode.DoubleRow

    xt_d = nc.dram_tensor("xt", [P, IT, t_tok], bft, kind="ExternalInput").ap()
    wq_d = nc.dram_tensor("wq", [P, nh, IT, P], bft, kind="ExternalInput").ap()
    wk_d = nc.dram_tensor("wk", [P, nh, IT, P], bft, kind="ExternalInput").ap()
    wv_d = nc.dram_tensor("wv", [P, IT, h_dim], bft, kind="ExternalInput").ap()
    wo_d = nc.dram_tensor("wo", [P, IT, h_dim], bft, kind="ExternalInput").ap()
    cos_d = nc.dram_tensor("cos", [P, 512], f32, kind="ExternalInput").ap()
    sin_d = nc.dram_tensor("sin", [P, 512], f32, kind="ExternalInput").ap()
    ones_d = nc.dram_tensor("ones", [P, P], bft, kind="ExternalInput").ap()
    OS_ = h_dim // 512
    wv8_d = nc.dram_tensor("wv8", [P, OS_, KF, 512], f8, kind="ExternalInput").ap()
    y_d = nc.dram_tensor("y", [t_tok, h_dim], bft, kind="ExternalOutput").ap()

    HH = P // 2

    with tile.TileContext(nc) as tc:
        with (
            tc.tile_pool(name="consts", bufs=1) as consts,
            tc.tile_pool(name="xtp", bufs=1) as xtp,
            tc.tile_pool(name="vp", bufs=1) as vp,
            tc.tile_pool(name="outp", bufs=1) as outp,
            tc.tile_pool(name="wqp", bufs=3) as wqp,
            tc.tile_pool(name="wkp", bufs=3) as wkp,
        ):
            # HAM warm-up: memset on the (empty) DVE queue — sub-us, vs
            # ~3 us on gpsimd — so the dummy matmuls start almost
            # immediately and ramp the PE clock inside the initial
            # DMA-wait window; real first matmuls otherwise spend ~3.6 us
            # at the 1.2 GHz mid p-state
            warm = consts.tile([P, 256], bft)
            nc.vector.memset(warm[:], 0.0)
            # 1/4 evac scale for the V phase (its PSUM runs at 4x: bf16
            # wv is x4 host-side so the fp8 DR term (8W)*(x/2) matches)
            qtr = consts.tile([P, 1], f32)
            nc.vector.memset(qtr[:], 0.25)

            # ones + the fp8 wv slices ride the (otherwise idle) GpSimd
            # SWDGE queue; cos/sin go on the scalar ring after the xt
            # stream (read only from the QK phase, ~110us in). x8 is cast
            # on-chip from xt by the idle DVE, so the fp8 V operands add
            # no load to the two busy HW rings during ov=0.
            ones_sb = consts.tile([P, P], bft)
            nc.gpsimd.dma_start(ones_sb[:], ones_d)
            cos_sb = consts.tile([P, 512], f32)
            sin_sb = consts.tile([P, 512], f32)
            x8_sb = consts.tile([P, KF, t_tok], f8)

            xt_sb = xtp.tile([P, IT, t_tok], bft)
            v_sb = vp.tile([P, TT, h_dim], bft)
            outT_sb = outp.tile([P, nh, t_tok], bft)

            # per-head QK weight prefetch, one head of lead
            def issue_head_w(h):
                wq_t = wqp.tile([P, IT, P], bft, name="wq_t")
                nc.scalar.dma_start(wq_t[:], wq_d[:, h])
                wk_t = wkp.tile([P, IT, P], bft, name="wk_t")
                nc.scalar.dma_start(wk_t[:], wk_d[:, h])
                return (wq_t, wk_t)

            # ---- V projection: v[t, o] ----
            # wv arrives as [P, 4, 512] "quad" DMAs (512 KB, one issue per 4
            # i-tiles) to stay under the DGE sequencer issue rate. ov=0 is
            # i-outer so xt streams in; ov>=1 are tt-outer so each PSUM
            # bank's WAR slack is a full 16-matmul group.
            with (
                tc.tile_pool(name="wvp", bufs=3) as wvp,
                tc.tile_pool(name="wv8p", bufs=2) as wv8p,
                tc.tile_pool(name="vps", bufs=1, space="PSUM") as vps,
            ):
                def evac_v(tt, ov, pv_t):
                    # ACT-only: a DVE evacuation here can queue behind the
                    # first head's rope ops (which wait on shift DMAs) and
                    # stall the PSUM-bank reuse chain at the phase boundary.
                    # scale 0.25 undoes the V-phase 4x PSUM convention.
                    nc.scalar.activation(
                        v_sb[:, tt, ov * 512:(ov + 1) * 512], pv_t[:], AF.Copy,
                        scale=qtr[:],
                    )

                def dr_v(tt, w8, pv_t):
                    # fp8 DoubleRow passes over k-tiles 0..KF-1 close the
                    # accumulation group (2 k-tiles per 512-cycle matmul)
                    for j in range(KF // 2):
                        nc.tensor.matmul(
                            pv_t[:],
                            x8_sb[:, 2 * j:2 * j + 2, tt * P:(tt + 1) * P],
                            w8[:, 2 * j:2 * j + 2],
                            start=False, stop=(j == KF // 2 - 1),
                            perf_mode=DRMODE,
                        )

                def wv_quad(q, ov):
                    w = wvp.tile([P, 4, 512], bft, name=f"wvq{q}")
                    nc.sync.dma_start(
                        w[:], wv_d[:, 4 * q:4 * q + 4, ov * 512:(ov + 1) * 512]
                    )
                    return w

                # fp8 wv slices ride the tail of the sync ring (slice 0 is
                # only read ~32us in, after the whole bf16 ov=0 block)
                def wv8_slice(ov):
                    w = wv8p.tile([P, KF, 512], f8, name="wv8s")
                    nc.sync.dma_start(w[:], wv8_d[:, ov])
                    return w

                # xt stream on scalar: the bf16 i-loop starts at i=4; the
                # fp8-covered tiles 0/1 are interleaved into the stream's
                # slack (the DVE casts below need them by ~+28us) and 2/3
                # ride the sync tail
                for i in (4, 6, 7, 8, 9, 10, 0, 11, 12, 1, 13, 14, 15):
                    nc.scalar.dma_start(xt_sb[:, i], xt_d[:, i])
                nc.scalar.dma_start(cos_sb[:], cos_d)
                nc.scalar.dma_start(sin_sb[:], sin_d)

                # ov = 0, i-outer; the first quad is split small-to-large so
                # the first matmul only waits on a 128 KB transfer
                pv = [vps.tile([P, 512], f32, name=f"pv{tt}") for tt in range(TT)]
                for _ in range(36):
                    nc.tensor.matmul(
                        pv[TT - 1][:, 0:256], warm[:, 0:128], warm[:],
                        start=True, stop=True,
                    )
                wv0a = wvp.tile([P, 512], bft, name="wv0a")
                nc.sync.dma_start(wv0a[:], wv_d[:, 4, 0:512])
                nc.sync.dma_start(xt_sb[:, 5], xt_d[:, 5])
                wv0b = wvp.tile([P, 512], bft, name="wv0b")
                nc.sync.dma_start(wv0b[:], wv_d[:, 5, 0:512])
                wv0c = wvp.tile([P, 2, 512], bft, name="wv0c")
                nc.sync.dma_start(wv0c[:], wv_d[:, 6:8, 0:512])
                wv_cur = [None, None] + [wv_quad(q, 0) for q in range(2, 4)]
                # wv8 slice 0 ahead of xt2/3: the ov=0 DR tail reads it at
                # ~+31us, while the x8 casts of tiles 2/3 (feeding the
                # second DR matmul, ~+32us) tolerate the later xt arrival
                wv8_cur = [wv8_slice(0)]
                nc.sync.dma_start(xt_sb[:, 2], xt_d[:, 2])
                nc.sync.dma_start(xt_sb[:, 3], xt_d[:, 3])
                wv8_cur.append(wv8_slice(1))
                # x8 = e4m3(xt/2), cast on the otherwise-idle DVE (must be
                # issued after ALL xt tile writes above, in program order)
                for i in range(KF):
                    nc.vector.tensor_scalar_mul(
                        x8_sb[:, i], xt_sb[:, i], 0.5
                    )
                wv_head = [wv0a[:], wv0b[:], wv0c[:, 0], wv0c[:, 1]]
                wv_next = []
                for i in range(4, IT):
                    # ov=1's quads prefetched in the second half of ov=0
                    if i in (8, 10, 12):
                        wv_next.append(wv_quad((i - 8) // 2 + 1, 1))
                    wsrc = wv_head[i - 4] if i < 8 else wv_cur[i // 4][:, i % 4]
                    for tt in range(TT):
                        nc.tensor.matmul(
                            pv[tt][:],
                            xt_sb[:, i, tt * P:(tt + 1) * P],
                            wsrc,
                            start=(i == 4),
                            stop=False,
                        )
                for tt in range(TT):
                    dr_v(tt, wv8_cur[0], pv[tt])
                    evac_v(tt, 0, pv[tt])
                for ov in range(1, OS):
                    wv_cur = [None] + wv_next
                    wv_next = []
                    if ov + 1 < OS:
                        wv8_cur.append(wv8_slice(ov + 1))
                    for tt in range(TT):
                        pv_t = vps.tile([P, 512], f32, name=f"pv{tt}")
                        for i in range(4, IT):
                            nc.tensor.matmul(
                                pv_t[:],
                                xt_sb[:, i, tt * P:(tt + 1) * P],
                                wv_cur[i // 4][:, i % 4],
                                start=(i == 4),
                                stop=False,
                            )
                        dr_v(tt, wv8_cur[ov], pv_t)
                        evac_v(tt, ov, pv_t)
                        if ov + 1 < OS and tt % 2 == 1 and tt // 2 >= 1:
                            wv_next.append(wv_quad(tt // 2, ov + 1))

            pend_w = [issue_head_w(0), issue_head_w(1)]

            # ---- per-head QK projection + RoPE + attention, interleaved ----
            with (
                tc.tile_pool(name="wop", bufs=2) as wop,
                tc.tile_pool(name="ysb", bufs=4) as ysb,
            ):
              with (
                  tc.tile_pool(name="ropep", bufs=4) as ropep,
                  tc.tile_pool(name="cbp", bufs=4) as cbp,
                  tc.tile_pool(name="mp", bufs=3) as mp,
                  tc.tile_pool(name="ep", bufs=4) as ep,
                  tc.tile_pool(name="rsp", bufs=2) as rsp,
                  tc.tile_pool(name="qkps", bufs=4, space="PSUM") as qkps,
                  tc.tile_pool(name="sps", bufs=2, space="PSUM") as sps,
                  tc.tile_pool(name="accps", bufs=2, space="PSUM") as accps,
              ):
                  def proj_group(w_t, cos_sb, sin_sb, rope, ts2):
                      sl = slice(ts2 * 512, (ts2 + 1) * 512)
                      pq = qkps.tile([P, 512], f32, name="pq")
                      for i in range(IT):
                          nc.tensor.matmul(
                              pq[:],
                              w_t[:, i],
                              xt_sb[:, i, sl],
                              start=(i == 0),
                              stop=(i == IT - 1),
                          )
                      qbf = cbp.tile([P, 512], bft, name="qbf")
                      nc.scalar.activation(qbf[:], pq[:], AF.Copy)
                      # rotate_half via SBUF->SBUF partition-shift DMAs, one
                      # half per HWDGE ring so they issue in parallel
                      # (sign is folded into the sin tables host-side)
                      rq = cbp.tile([P, 512], bft, name="rq")
                      nc.sync.dma_start(rq[0:HH, :], qbf[HH:P, :])
                      nc.scalar.dma_start(rq[HH:P, :], qbf[0:HH, :])
                      m1 = mp.tile([P, 512], f32, name="m1")
                      nc.vector.tensor_tensor(m1[:], pq[:], cos_sb[:], AluOpType.mult)
                      m2 = mp.tile([P, 512], f32, name="m2")
                      nc.vector.tensor_tensor(m2[:], rq[:], sin_sb[:], AluOpType.mult)
                      nc.vector.tensor_tensor(rope[:, sl], m1[:], m2[:], AluOpType.add)

                  def attn_scores(ropes, b):
                      q_rope, k_rope = ropes
                      bs = slice(b * s_len, (b + 1) * s_len)
                      pS = sps.tile([P, SK, s_len], f32, name="pS")
                      for sk in range(SK):
                          nc.tensor.matmul(
                              pS[:, sk],
                              k_rope[:, b * s_len + sk * P: b * s_len + (sk + 1) * P],
                              q_rope[:, bs],
                              start=True,
                              stop=True,
                          )
                      ebf = ep.tile([P, SK, s_len], bft, name="ebf")
                      nc.scalar.activation(ebf[:], pS[:], AF.Exp)
                      return ebf

                  def attn_tail(h, b, ebf):
                      bs = slice(b * s_len, (b + 1) * s_len)
                      # sums and attnV share one PSUM bank (halves of acc)
                      acc = accps.tile([P, 2, s_len], f32, name="acc")
                      for sk in range(SK):
                          nc.tensor.matmul(
                              acc[:, 0], ones_sb[:], ebf[:, sk],
                              start=(sk == 0), stop=(sk == SK - 1),
                          )
                      rsb = rsp.tile([P, s_len], f32, name="rsb")
                      nc.vector.reciprocal_approx_fast(rsb[:], acc[:, 0])
                      for sk in range(SK):
                          nc.tensor.matmul(
                              acc[:, 1],
                              v_sb[:, SK * b + sk, h * P:(h + 1) * P],
                              ebf[:, sk],
                              start=(sk == 0), stop=(sk == SK - 1),
                          )
                      nc.vector.tensor_tensor(
                          outT_sb[:, h, bs], acc[:, 1], rsb[:], AluOpType.mult
                      )

                  wo_tiles = {}
                  prev = None
                  for h in range(nh):
                      wq_t, wk_t = pend_w[h]
                      if h + 2 < nh:
                          pend_w.append(issue_head_w(h + 2))
                      if h == nh - 2:
                          # wo slice 0 prefetch: all 16 issues at the top of
                          # head 14, ahead of that head's shift DMAs on sync
                          for quarter in range(4):
                              wo_tiles.setdefault(0, []).extend(
                                  _issue_wo_quarter(nc, wop, wo_d, 0, quarter, bft)
                              )
                      q_rope = ropep.tile([P, t_tok], bft, name="q_rope")
                      k_rope = ropep.tile([P, t_tok], bft, name="k_rope")
                      proj_group(wq_t, cos_sb, sin_sb, q_rope, 0)
                      proj_group(wq_t, cos_sb, sin_sb, q_rope, 1)
                      ebfs = None
                      if prev is not None:
                          ebfs = [attn_scores(prev[1], b) for b in range(b_loc)]
                      proj_group(wk_t, cos_sb, sin_sb, k_rope, 0)
                      if prev is not None:
                          attn_tail(prev[0], 0, ebfs[0])
                          attn_tail(prev[0], 1, ebfs[1])
                      proj_group(wk_t, cos_sb, sin_sb, k_rope, 1)
                      if prev is not None:
                          attn_tail(prev[0], 2, ebfs[2])
                          attn_tail(prev[0], 3, ebfs[3])
                      prev = (h, (q_rope, k_rope))

                  # ---- output projection groups (PSUM banks from qkps,
                  # which has no projection work left) ----
                  def o_group(o2, tt):
                      wos = wo_tiles[o2]
                      py = qkps.tile([P, 512], f32, name="pq")
                      for o in range(IT):
                          nc.tensor.matmul(
                              py[:],
                              outT_sb[:, o, tt * P:(tt + 1) * P],
                              wos[o // 4][:, o % 4],
                              start=(o == 0),
                              stop=(o == IT - 1),
                          )
                      y_t = ysb.tile([P, 512], bft, name="y_t")
                      if o2 == OS - 1 and tt == TT - 1:
                          # final tile: halves in parallel on both engines
                          # and both rings to shorten the drain tail
                          nc.scalar.activation(y_t[:, 0:256], py[:, 0:256], AF.Copy)
                          nc.vector.tensor_copy(y_t[:, 256:512], py[:, 256:512])
                          nc.sync.dma_start(
                              y_d[tt * P:(tt + 1) * P,
                                  o2 * 512:o2 * 512 + 256], y_t[:, 0:256]
                          )
                          nc.scalar.dma_start(
                              y_d[tt * P:(tt + 1) * P,
                                  o2 * 512 + 256:(o2 + 1) * 512], y_t[:, 256:512]
                          )
                      else:
                          if tt % 2 == 0:
                              nc.scalar.activation(y_t[:], py[:], AF.Copy)
                          else:
                              nc.vector.tensor_copy(y_t[:], py[:])
                          nc.sync.dma_start(
                              y_d[tt * P:(tt + 1) * P, o2 * 512:(o2 + 1) * 512],
                              y_t[:],
                          )
                      # next wo slice, 4 issues at a time between copies
                      if o2 + 1 < OS and tt < 4:
                          wo_tiles.setdefault(o2 + 1, []).extend(
                              _issue_wo_quarter(
                                  nc, wop, wo_d, o2 + 1, tt, bft, engine=nc.scalar
                              )
                          )

                  # trailing attention for the last head, interleaved into
                  # the first O slice: tails for batch b complete outT for
                  # token tiles 2b, 2b+1, unblocking those o-groups
                  ebfs = [attn_scores(prev[1], b) for b in range(b_loc)]
                  attn_tail(prev[0], 0, ebfs[0])
                  attn_tail(prev[0], 1, ebfs[1])
                  o_group(0, 0)
                  o_group(0, 1)
                  attn_tail(prev[0], 2, ebfs[2])
                  o_group(0, 2)
                  o_group(0, 3)
                  attn_tail(prev[0], 3, ebfs[3])
                  for tt in range(4, TT):
                      o_group(0, tt)
                  for o2 in range(1, OS):
                      for tt in range(TT):
                          o_group(o2, tt)

    nc.compile()
    return nc


def _issue_wo_quarter(nc, wop, wo_d, o2, quarter, bft, engine=None):
    t = wop.tile([P, 4, 512], bft, name=f"woq{quarter}")
    (engine or nc.sync).dma_start(
        t[:], wo_d[:, 4 * quarter:4 * quarter + 4, o2 * 512:(o2 + 1) * 512]
    )
    return [t]


def _host_prep(hidden_states, Wq, Wk, Wv, Wo):
    """Host-side sharding + layout prep. Returns per-core in_maps."""
    x = np.asarray(hidden_states, dtype=np.float32).reshape(B * S, H)

    # weights: transposed + tiled layouts, cast to bf16
    WqT = np.ascontiguousarray(np.asarray(Wq).T)   # [i, o]
    WkT = np.ascontiguousarray(np.asarray(Wk).T)
    WvT = np.ascontiguousarray(np.asarray(Wv).T)
    WoT = np.ascontiguousarray(np.asarray(Wo).T)
    IT = H // P
    # per-head column blocks: [P(p), nh, IT, P(o-within-head)]
    # 1/sqrt(hd) attention scale folded into Wq host-side
    scale = np.float32(HD ** -0.5)
    wq_h = np.ascontiguousarray(
        (WqT * scale).reshape(IT, P, NH, HD).transpose(1, 2, 0, 3)
    ).astype(bf16)
    wk_h = np.ascontiguousarray(
        WkT.reshape(IT, P, NH, HD).transpose(1, 2, 0, 3)
    ).astype(bf16)
    # plain i-tiled: [P, IT, H]. wv carries x4 (the V-phase PSUM runs at
    # 4x so the fp8 DR term (8W)*(x/2) matches; evac rescales by 1/4)
    wv_h = np.ascontiguousarray(
        (WvT * 4.0).reshape(IT, P, H).transpose(1, 0, 2)
    ).astype(bf16)
    wo_h = np.ascontiguousarray(WoT.reshape(IT, P, H).transpose(1, 0, 2)).astype(bf16)
    # fp8 DoubleRow weights for V k-tiles 0..KF-1, scaled x8 to dodge both
    # e4m3 saturation and the HW subnormal flush; ov-major layout so each
    # [P, KF, 512] slice is DMA-contiguous per partition
    wv8_h = np.ascontiguousarray(
        (WvT[0:KF * P] * 8.0).reshape(KF, P, H // 512, 512).transpose(1, 2, 0, 3)
    ).astype(ml_dtypes.float8_e4m3)

    cos, sin = _rope_tables_np(S, HD)              # [s, d]
    cosT = np.ascontiguousarray(cos.T)             # [d, s]
    sinT = np.ascontiguousarray(sin.T)
    # rotate-half sign folded into sin: rq[d] = q[(d+64)%128], sign -1 for d<64
    sgn = np.where(np.arange(HD) < HD // 2, -1.0, 1.0).astype(np.float32)[:, None]
    sinT = sinT * sgn
    cos = np.tile(cosT, (1, 2)).astype(np.float32)   # [128, 512]
    sin = np.tile(sinT, (1, 2)).astype(np.float32)
    ones = np.ones((P, P), np.float32).astype(bf16)

    shared = {
        "wq": wq_h, "wk": wk_h, "wv": wv_h, "wo": wo_h, "wv8": wv8_h,
        "cos": cos, "sin": sin,
        "ones": ones,
    }
    in_maps = []
    for c in range(N_CORES):
        xc = x[c * T:(c + 1) * T]                   # [T, H]
        xTc = np.ascontiguousarray(xc.T).astype(bf16)  # [H, T]
        xt = np.ascontiguousarray(
            xTc.reshape(IT, P, T).transpose(1, 0, 2)
        )                                           # [P, IT, T]
        in_maps.append({"xt": xt, **shared})
    return in_maps


def _run(hidden_states, Wq, Wk, Wv, Wo, **spmd_kwargs):
    from concourse import bass_utils

    if "nc" not in _CACHE:
        _CACHE["nc"] = build_nc()
    nc = _CACHE["nc"]

    in_maps = _host_prep(hidden_states, Wq, Wk, Wv, Wo)
    res = bass_utils.run_bass_kernel_spmd(
        nc, in_maps, core_ids=list(range(N_CORES)), **spmd_kwargs
    )
    y = np.concatenate([r["y"] for r in res.results], axis=0)  # [B*S, H]
    return y.reshape(B, S, H).astype(np.float32), res


def kernel(hidden_states, Wq, Wk, Wv, Wo):
    y, _ = _run(hidden_states, Wq, Wk, Wv, Wo)
    return y


def run_traced(hidden_states, Wq, Wk, Wv, Wo):
    """Like kernel(), but captures an NTFF profile; returns (y, BassKernelResults)."""
    return _run(hidden_states, Wq, Wk, Wv, Wo, trace=True)

